# revision 1
# baseline (speedup 1.0000x reference)
"""Trainium2 Bass kernel for nn_CooperationModule (MoE-style expert sum).

Math (reference):
    pre[b, e, h] = (x[b, :] - c[e, :]) @ W[e, h, :] + bias[e, h]
    out[b, h]    = sum_e relu(pre[b, e, h])

Sharding: batch-parallel across 8 NeuronCores (B=4096 -> 512 rows/core).
Each core holds all 16 experts' weights and computes the full expert sum
for its batch shard -- no collectives needed (an expert-parallel AllReduce
of the 32MB output would cost ~350us, far more than the extra W reads).

Per-core compute layout (h on partitions so bias/relu fuse on ScalarE):
    for e in 0..15:
        xe[d, b]   = xT[d, b] - c[e, d]          (DVE tensor_scalar_sub, per-partition scalar)
        for ht in 0..15:
            psum[h128, b512] = sum_ki WT_e[d128, h128].T @ xe[d128, b512]   (4 matmuls)
            t = relu(psum + bias_e[h128])        (ScalarE activation, per-partition bias)
            acc[ht] += t                         (DVE tensor_add; e==0 writes directly)
    out_t[h, b] = acc                            (DMA out; host transposes)
"""

import os
import sys

import numpy as np

sys.path.insert(0, "/opt/trn_rl_repo")

import concourse.bass as bass
import concourse.mybir as mybir
import concourse.tile as tile
from concourse import bacc
from concourse.bass_utils import run_bass_kernel_spmd

B, E, D, H = 4096, 16, 512, 2048
NCORES = 8
BL = B // NCORES  # 512 batch rows per core
P = 128
DT = D // P  # 4 contraction tiles
HT = H // P  # 16 output-partition tiles

# matmul input dtype: "f32r" (full-rate fp32-reduced), "f32" (4x slower), "bf16"
MM_DTYPE = os.environ.get("KERNEL_MM_DTYPE", "f32r")

_cache = {}


def _build(nc_dtype_key, reps=1):
    nc = bacc.Bacc(None, target_bir_lowering=False)

    mm_dt = {
        "f32r": mybir.dt.float32r,
        "f32": mybir.dt.float32,
        "bf16": mybir.dt.bfloat16,
    }[nc_dtype_key]

    # DRAM layouts are pre-baked on the host to match the SBUF tiles exactly,
    # so every load is one contiguous-per-partition DMA.
    xt = nc.declare_dram_parameter("xt", [P, DT, BL], mybir.dt.float32, isOutput=False)
    # W is stored in DRAM in the matmul dtype (float32r is bit-identical to f32)
    wt = nc.declare_dram_parameter("wt", [E, D, H], mm_dt, isOutput=False)
    ct = nc.declare_dram_parameter("ct", [P, DT, E], mybir.dt.float32, isOutput=False)
    bt = nc.declare_dram_parameter("bt", [P, HT, E], mybir.dt.float32, isOutput=False)
    out_t = nc.declare_dram_parameter("out_t", [H, BL], mybir.dt.float32, isOutput=True)

    with tile.TileContext(nc) as tc:
        with (
            tc.tile_pool(name="singles", bufs=1) as singles,
            tc.tile_pool(name="wpool", bufs=2) as wpool,
            tc.tile_pool(name="xepool", bufs=2) as xepool,
            tc.tile_pool(name="tpool", bufs=4) as tpool,
            tc.tile_pool(name="accpool", bufs=1) as accpool,
            tc.tile_pool(name="psum", bufs=8, space="PSUM") as psum_pool,
        ):
            # --- one-time loads (small; SWDGE keeps the wait fanout low) --------
            # xT: [128, DT, BL]; element [p, ki, b] = x[b, ki*128+p]
            xt_all = singles.tile([P, DT, BL], mybir.dt.float32, name="xt_all")
            nc.gpsimd.dma_start(out=xt_all, in_=xt[:, :, :])
            xt_sb = [xt_all[:, ki, :] for ki in range(DT)]

            # centers^T: [128, DT, E]; element [p, ki, e] = c[e, ki*128+p]
            ct_sb = singles.tile([P, DT, E], mybir.dt.float32, name="ct_sb")
            nc.gpsimd.dma_start(out=ct_sb, in_=ct[:, :, :])

            # bias^T: [128, HT, E]; element [p, ht, e] = bias[e, ht*128+p]
            bt_sb = singles.tile([P, HT, E], mybir.dt.float32, name="bt_sb")
            nc.gpsimd.dma_start(out=bt_sb, in_=bt[:, :, :])

            # persistent accumulators: [128, BL] per ht
            acc = []
            for ht in range(HT):
                a = accpool.tile([P, BL], mybir.dt.float32, name=f"acc{ht}")
                acc.append(a)

            # --- main loop (reps>1 only for timing: amortizes dispatch cost) ----
            for _rep in range(reps):
              for e in range(E):
                # W^T tiles for this expert: [128, H] per ki
                w_sb = []
                for ki in range(DT):
                    w_tile = wpool.tile(
                        [P, H], mm_dt, name=f"w{ki}", tag=f"w{ki}"
                    )
                    nc.sync.dma_start(
                        out=w_tile, in_=wt[e, ki * P : (ki + 1) * P, :]
                    )
                    w_sb.append(w_tile)

                # xe = xT - c_e (broadcast per-partition scalar along free dim)
                xe_sb = []
                for ki in range(DT):
                    xe_tile = xepool.tile(
                        [P, BL], mm_dt, name=f"xe{ki}", tag=f"xe{ki}"
                    )
                    nc.vector.tensor_scalar_sub(
                        xe_tile, xt_sb[ki], ct_sb[:, ki, e : e + 1]
                    )
                    xe_sb.append(xe_tile)

                for ht in range(HT):
                    ps = psum_pool.tile([P, BL], mybir.dt.float32, name="ps", tag="ps")
                    for ki in range(DT):
                        nc.tensor.matmul(
                            ps,
                            w_sb[ki][:, ht * P : (ht + 1) * P],
                            xe_sb[ki],
                            start=(ki == 0),
                            stop=(ki == DT - 1),
                        )
                    bias_ap = bt_sb[:, ht, e : e + 1]
                    if e == 0:
                        nc.scalar.activation(
                            acc[ht], ps, mybir.ActivationFunctionType.Relu,
                            bias=bias_ap, scale=1.0,
                        )
                    else:
                        t = tpool.tile([P, BL], mybir.dt.float32, name="t", tag="t")
                        nc.scalar.activation(
                            t, ps, mybir.ActivationFunctionType.Relu,
                            bias=bias_ap, scale=1.0,
                        )
                        nc.vector.tensor_add(acc[ht], acc[ht], t)

              # --- store --------------------------------------------------------
              for ht in range(HT):
                nc.sync.dma_start(
                    out=out_t[ht * P : (ht + 1) * P, :], in_=acc[ht]
                )

    nc.finalize()
    return nc


def _get_nc(reps=1):
    key = (MM_DTYPE, reps)
    if key not in _cache:
        _cache[key] = _build(MM_DTYPE, reps)
    return _cache[key]


def make_in_maps(semantic_vec, field_centers, W, b):
    # Host-side relayout (layout prep only; all math runs on device).
    # xt[p, ki, b] = x[b, ki*128 + p]
    xt_full = np.ascontiguousarray(
        semantic_vec.astype(np.float32).T.reshape(DT, P, B).transpose(1, 0, 2)
    )  # [P, DT, B]
    wt_full = np.ascontiguousarray(W.transpose(0, 2, 1)).astype(np.float32)  # [E, D, H]
    # ct[p, ki, e] = c[e, ki*128 + p]
    ct_full = np.ascontiguousarray(
        field_centers.astype(np.float32).T.reshape(DT, P, E).transpose(1, 0, 2)
    )  # [P, DT, E]
    # bt[p, ht, e] = b[e, ht*128 + p]
    bt_full = np.ascontiguousarray(
        b.astype(np.float32).T.reshape(HT, P, E).transpose(1, 0, 2)
    )  # [P, HT, E]
    if MM_DTYPE == "bf16":
        import ml_dtypes

        wt_full = wt_full.astype(ml_dtypes.bfloat16)

    in_maps = []
    for k in range(NCORES):
        in_maps.append(
            {
                "xt": np.ascontiguousarray(xt_full[:, :, k * BL : (k + 1) * BL]),
                "wt": wt_full,
                "ct": ct_full,
                "bt": bt_full,
            }
        )
    return in_maps


def kernel(semantic_vec, field_centers, W, b, _want_trace=False):
    assert semantic_vec.shape == (B, D)
    assert W.shape == (E, H, D)

    nc = _get_nc()
    in_maps = make_in_maps(semantic_vec, field_centers, W, b)

    res = run_bass_kernel_spmd(
        nc, in_maps, core_ids=list(range(NCORES)), trace=_want_trace
    )

    out = np.empty((B, H), dtype=np.float32)
    for k in range(NCORES):
        out[k * BL : (k + 1) * BL, :] = res.results[k]["out_t"].T
    if _want_trace:
        return out, res
    return out

